# revision 37
# baseline (speedup 1.0000x reference)
"""Multi-head causal self-attention on 8 Trainium2 NeuronCores.

Problem: x[4,2048,1024] @ w_qkv[1024,3072] -> 16-head causal attention
         -> @ w_out[1024,1024] + b_out.

Sharding (hardcoded): 8 cores = 4 batches x 2 head-groups of 8 heads.
Core c handles batch b = c//2 and heads hg*8..hg*8+8, hg = c%2.
Each core computes a partial output [2048,1024] (its 8 heads pushed
through its w_out row-slice); host sums the two head-group partials per
batch and adds b_out.

Everything computes in fp16 (10 mantissa bits; fp32 PSUM accumulation),
which runs matmuls at full 1 cycle/row PE rate and lands ~7e-4 relative
error vs the fp32 reference.

Device algorithm per core (all "transposed orientation" so the only
transpose needed -- x^T -- is done for free on the host):
  qT/kT [512, 2048] and v (natural [2048, 512]) via fp16 matmuls.
  Per head pair (2 heads = 128 partitions), per 512-wide query chunk:
    scores^T[j,i] for both heads into one 2-bank PSUM tile via
    row-tiled (K=64) matmul pairs; ONE exp per key-tile on ScalarE
    (p^T fp16); causal masking via a precomputed 0/1 mask multiply on
    the diagonal band plus variable-width (narrowed) tiles;
    out^T[d,i] += col-tiled matmuls (PSUM accum over j),
    denom[i]   += ones-vector matmuls (M=1) into shared denom banks
    (4 col-strip rows per bank, zero-established by a dummy matmul).
  att^T (unnormalized) is copied to SBUF immediately (frees PSUM);
  1/denom via one batched DVE reciprocal per bank, broadcast over
  partitions via a DRAM bounce, then in-place multiply into att^T.
  partial = att^T.T @ w_out_slice -> DMA to DRAM.

Emission is software-pipelined per 512-token stage s: QKV(s),
out-projection(s-1), attention(s), so the Tile scheduler overlaps
PE-heavy projection work with ScalarE-heavy softmax work and hides the
softmax-denominator normalization latency.
"""

import os
import sys

import numpy as np

if "/opt/trn_rl_repo" not in sys.path:
    sys.path.insert(0, "/opt/trn_rl_repo")

B, T, C = 4, 2048, 1024
H, D = 16, 64
NCORES = 8
HPC = 8  # heads per core
PAIRS = 4  # head pairs per core
CCH = 8  # contraction chunks over C (1024/128)
ICH = 4  # i (query) chunks of 512
NJT = 16  # j (key) tiles of 128

_CACHE = {}


def _build_program():
    import concourse.mybir as mybir
    import concourse.tile as tile
    from concourse import bacc

    f32 = mybir.dt.float32
    f32r = mybir.dt.float32r
    bf16 = mybir.dt.bfloat16
    f16 = mybir.dt.float16
    EXP = mybir.ActivationFunctionType.Exp
    CPY = mybir.ActivationFunctionType.Copy

    nc = bacc.Bacc(
        "TRN2", target_bir_lowering=False, debug=False, num_devices=NCORES
    )
    xt = nc.dram_tensor("xt", [C, T], f16, kind="ExternalInput").ap()
    wq = nc.dram_tensor("wq", [C, 512], f16, kind="ExternalInput").ap()
    wk = nc.dram_tensor("wk", [C, 512], f16, kind="ExternalInput").ap()
    wv = nc.dram_tensor("wv", [C, 512], f16, kind="ExternalInput").ap()
    wo = nc.dram_tensor("wo", [512, C], f16, kind="ExternalInput").ap()
    msk = nc.dram_tensor("msk", [128, 896], f16, kind="ExternalInput").ap()
    out = nc.dram_tensor("out", [T, C], f32, kind="ExternalOutput").ap()

    with tile.TileContext(nc) as tc:
        with (
            tc.tile_pool(name="wpool", bufs=16) as wpool,
            tc.tile_pool(name="wvpool", bufs=8) as wvpool,
            tc.tile_pool(name="wopool", bufs=4) as wopool,
            tc.tile_pool(name="xpool", bufs=8) as xpool,
            tc.tile_pool(name="qkpool", bufs=8) as qkpool,
            tc.tile_pool(name="vpool", bufs=16) as vpool,
            tc.tile_pool(name="apool", bufs=4) as apool,
            tc.tile_pool(name="ppool", bufs=16) as ppool,
            tc.tile_pool(name="cpool", bufs=1) as cpool,
            tc.tile_pool(name="rpool", bufs=4) as rpool,
            tc.tile_pool(name="qpool", bufs=8) as qpool,
            tc.tile_pool(name="opool", bufs=4) as opool,
            tc.tile_pool(name="dpool", bufs=4, space="DRAM") as dpool,
            tc.tile_pool(name="ps_a", bufs=2, space="PSUM") as ps_a,
            tc.tile_pool(name="ps_s", bufs=2, space="PSUM") as ps_s,
            tc.tile_pool(name="ps_o", bufs=1, space="PSUM") as ps_o,
            tc.tile_pool(name="ps_d", bufs=1, space="PSUM") as ps_d,
        ):
            # ---- constants / weights resident in SBUF ----
            mask_sb = cpool.tile([128, 896], f16, name="mask_sb")
            nc.sync.dma_start(out=mask_sb, in_=msk)
            ones_sb = cpool.tile([128, 1], f16, name="ones_sb")
            nc.vector.memset(ones_sb, 1.0)
            zer_sb = cpool.tile([128, 128], f16, name="zer_sb")
            nc.vector.memset(zer_sb, 0.0)

            w_sb = {}

            def load_w(wname, wap):
                for cc in range(CCH):
                    t = wpool.tile(
                        [128, 512], f16, name=f"{wname}_{cc}", tag="w"
                    )
                    nc.sync.dma_start(
                        out=t, in_=wap[cc * 128 : (cc + 1) * 128, :]
                    )
                    w_sb[wname, cc] = t

            # First compute needs wq + x^T: queue those DMAs first so the
            # PE starts early. x^T loads as 8 big 512KB transfers (whole
            # token range per contraction chunk).
            load_w("wq", wq)
            xt_sb = []
            for cc in range(CCH):
                xt_t = xpool.tile([128, T], f16, name=f"xt_{cc}", tag="xt")
                nc.sync.dma_start(out=xt_t, in_=xt[cc * 128 : (cc + 1) * 128, :])
                xt_sb.append(xt_t)
            load_w("wk", wk)
            for cc in range(CCH):
                t = wvpool.tile([128, 512], f16, name=f"wv_{cc}", tag="wv")
                nc.sync.dma_start(out=t, in_=wv[cc * 128 : (cc + 1) * 128, :])
                w_sb["wv", cc] = t
            wo_sb = []
            for fc in range(4):
                t = wopool.tile([128, C], f16, name=f"wo_{fc}", tag="wo")
                nc.sync.dma_start(out=t, in_=wo[fc * 128 : (fc + 1) * 128, :])
                wo_sb.append(t)

            # ---- persistent activations ----
            qT = [
                qkpool.tile([128, T], f16, name=f"qT_{p}", tag="qk")
                for p in range(PAIRS)
            ]
            kT = [
                qkpool.tile([128, T], f16, name=f"kT_{p}", tag="qk")
                for p in range(PAIRS)
            ]
            v_sb = [
                vpool.tile([128, 512], f16, name=f"v_{j}", tag="v")
                for j in range(NJT)
            ]
            att = [
                apool.tile([128, T], f16, name=f"att_{p}", tag="att")
                for p in range(PAIRS)
            ]

            def phase_a(t4):
                """QKV projections for token chunk t4 (512 tokens)."""
                tsl4 = slice(t4 * 512, (t4 + 1) * 512)
                xts = [xt_sb[cc][:, tsl4] for cc in range(CCH)]
                for wname, dst in (("wq", qT), ("wk", kT)):
                    for n in range(PAIRS):
                        ps = ps_a.tile([128, 512], f32, name="ps_qk", tag="psA")
                        for cc in range(CCH):
                            nc.tensor.matmul(
                                ps,
                                lhsT=w_sb[wname, cc][:, n * 128 : (n + 1) * 128],
                                rhs=xts[cc][:],
                                start=(cc == 0),
                                stop=(cc == CCH - 1),
                            )
                        nc.vector.tensor_copy(
                            dst[n][:, t4 * 512 : (t4 + 1) * 512], ps
                        )
                for tt in range(4):
                    ps = ps_a.tile([128, 512], f32, name="ps_v", tag="psA")
                    for cc in range(CCH):
                        nc.tensor.matmul(
                            ps,
                            lhsT=xts[cc][:, tt * 128 : (tt + 1) * 128],
                            rhs=w_sb["wv", cc][:],
                            start=(cc == 0),
                            stop=(cc == CCH - 1),
                        )
                    nc.vector.tensor_copy(v_sb[t4 * 4 + tt], ps)

            def phase_b(ic):
                """Attention for query chunk ic (512 queries), all pairs."""
                isl = slice(ic * 512, (ic + 1) * 512)
                njt = 4 * ic + 4
                # Two denominator banks per ic: bank A rows {0,32,64,96} =
                # pairs 0,1; bank B = pairs 2,3. One zeroing matmul each
                # establishes the group and write-ordering.
                dbanks = []
                for g in range(2):
                    bank = ps_d.tile([128, 512], f32, name=f"ps_den{g}", tag="psd")
                    nc.tensor.matmul(
                        bank,
                        lhsT=zer_sb,
                        rhs=mask_sb[:, 0:512],
                        start=True,
                        stop=False,
                        skip_group_check=True,
                    )
                    dbanks.append(bank)

                def norm_group(g):
                    """1/denominators for pairs 2g, 2g+1 -> rdb + in-place mul."""
                    bank = dbanks[g]
                    rec = rpool.tile([128, 1024], f32, name="rec", tag="rec")
                    nc.vector.tensor_copy(rec[0:97, 0:512], bank[0:97, :])
                    nc.vector.reciprocal(rec[0:97, 512:1024], rec[0:97, 0:512])
                    dsc = dpool.tile([4, 512], f32, name="dsc", tag="dsc")
                    for r in range(4):
                        nc.sync.dma_start(
                            out=dsc[r : r + 1, :],
                            in_=rec[32 * r : 32 * r + 1, 512:1024],
                        )
                    for lp in range(2):
                        pr = 2 * g + lp
                        rdb = rpool.tile([128, 512], f32, name="rdb", tag="rdb")
                        nc.sync.dma_start(
                            out=rdb[0:64, :],
                            in_=dsc[2 * lp : 2 * lp + 1, :].broadcast_to([64, 512]),
                        )
                        nc.sync.dma_start(
                            out=rdb[64:128, :],
                            in_=dsc[2 * lp + 1 : 2 * lp + 2, :].broadcast_to(
                                [64, 512]
                            ),
                        )
                        asl = att[pr][:, isl]
                        nc.vector.tensor_mul(asl, asl, rdb)

                for pr in range(PAIRS):
                    ps_out = ps_o.tile([128, 512], f32, name="ps_out", tag="pso")
                    dbank = dbanks[pr // 2]
                    dp0 = 64 * (pr % 2)
                    dp1 = 64 * (pr % 2) + 32
                    # Zero the whole ps_out bank in one matmul: establishes
                    # the accumulation group and a WAW dep ordering it before
                    # both col-tiled sub-chains.
                    nc.tensor.matmul(
                        ps_out,
                        lhsT=zer_sb,
                        rhs=mask_sb[:, 0:512],
                        start=True,
                        stop=False,
                        skip_group_check=True,
                    )
                    pacc0 = qpool.tile([128, 512], f16, name="pacc0", tag="pacc")
                    pacc1 = qpool.tile([128, 512], f16, name="pacc1", tag="pacc")
                    for jt in range(njt):
                        jsl = slice(jt * 128, (jt + 1) * 128)
                        dpos = jt - 4 * ic
                        # Causal: query columns below 128*dpos within this
                        # chunk see none of this key tile. fp32r matmuls run
                        # 4 cyc/row under N=256, so don't narrow QK below
                        # that; bf16 pv/den and the exp narrow fully.
                        ioff = 128 * dpos if dpos > 0 else 0
                        qoff = min(ioff, 256)
                        w = 512 - ioff
                        islw = slice(ic * 512 + ioff, (ic + 1) * 512)
                        islq = slice(ic * 512 + qoff, (ic + 1) * 512)
                        sb = ps_s.tile([128, 1024], f32, name="sb", tag="pss")
                        nc.tensor.matmul(
                            sb[:, qoff:512],
                            lhsT=kT[pr][0:64, jsl],
                            rhs=qT[pr][0:64, islq],
                            start=True,
                            stop=True,
                            tile_position=(0, 0),
                        )
                        nc.tensor.matmul(
                            sb[:, 512:1024],
                            lhsT=kT[pr][64:128, jsl],
                            rhs=qT[pr][64:128, isl],
                            start=True,
                            stop=True,
                            tile_position=(64, 0),
                        )
                        pTb = ppool.tile([128, 1024], f16, name="pTb", tag="pT")
                        # One exp covers both heads; the dead zone between
                        # the halves on diagonal tiles holds stale-but-finite
                        # scores and is never read downstream.
                        nc.scalar.activation(
                            pTb[:, ioff:1024], sb[:, ioff:1024], EXP, scale=0.125
                        )
                        pT0 = pTb[:, 0:512]
                        pT1 = pTb[:, 512:1024]
                        if dpos >= 0:
                            msl = mask_sb[:, 384 : 384 + w]
                            nc.vector.tensor_mul(
                                pT0[:, ioff:512], pT0[:, ioff:512], msl
                            )
                            nc.vector.tensor_mul(
                                pT1[:, ioff:512], pT1[:, ioff:512], msl
                            )
                        last = jt == njt - 1
                        vt = v_sb[jt]
                        nc.tensor.matmul(
                            ps_out[0:64, ioff:512],
                            lhsT=vt[:, pr * 128 : pr * 128 + 64],
                            rhs=pT0[:, ioff:512],
                            start=False,
                            stop=False,
                            tile_position=(0, 0),
                            skip_group_check=True,
                        )
                        nc.tensor.matmul(
                            ps_out[64:128, ioff:512],
                            lhsT=vt[:, pr * 128 + 64 : pr * 128 + 128],
                            rhs=pT1[:, ioff:512],
                            start=False,
                            stop=last,
                            tile_position=(0, 64),
                            skip_group_check=True,
                        )
                        if jt == 0:
                            nc.vector.tensor_copy(pacc0, pT0)
                            nc.vector.tensor_copy(pacc1, pT1)
                        else:
                            nc.vector.tensor_add(
                                pacc0[:, ioff:512],
                                pacc0[:, ioff:512],
                                pT0[:, ioff:512],
                            )
                            nc.vector.tensor_add(
                                pacc1[:, ioff:512],
                                pacc1[:, ioff:512],
                                pT1[:, ioff:512],
                            )
                    # Partition-reduce the accumulated p-sums into the
                    # shared denominator bank (2 matmuls instead of 2/key-tile).
                    nc.tensor.matmul(
                        dbank[dp0 : dp0 + 1, :],
                        lhsT=ones_sb,
                        rhs=pacc0,
                        start=False,
                        stop=False,
                        tile_position=(0, dp0),
                        skip_group_check=True,
                    )
                    nc.tensor.matmul(
                        dbank[dp1 : dp1 + 1, :],
                        lhsT=ones_sb,
                        rhs=pacc1,
                        start=False,
                        stop=False,
                        tile_position=(0, dp1),
                        skip_group_check=True,
                    )
                    # Unnormalized copy frees ps_out quickly; normalization
                    # happens in-place on att once the broadcast lands.
                    asl = att[pr][:, isl]
                    nc.vector.tensor_copy(asl, ps_out)
                    if pr % 2 == 1:
                        norm_group(pr // 2)

            def phase_c(s):
                """Output projection for token tiles 4s..4s+4."""
                for tt in range(4 * s, 4 * s + 4):
                    tsl = slice(tt * 128, (tt + 1) * 128)
                    for n in range(2):
                        ps = ps_a.tile([128, 512], f32, name="ps_c", tag="psA")
                        for fc in range(4):
                            nc.tensor.matmul(
                                ps,
                                lhsT=att[fc][:, tsl],
                                rhs=wo_sb[fc][:, n * 512 : (n + 1) * 512],
                                start=(fc == 0),
                                stop=(fc == 3),
                            )
                        ost = opool.tile([128, 512], f32, name="ost", tag="ost")
                        nc.scalar.activation(ost, ps, CPY)
                        nc.sync.dma_start(
                            out=out[tsl, n * 512 : (n + 1) * 512], in_=ost
                        )

            for s in range(4):
                phase_a(s)
                if s >= 1:
                    phase_c(s - 1)
                phase_b(s)
            phase_c(3)

    nc.compile()
    return nc


def _get_program():
    if "nc" not in _CACHE:
        _CACHE["nc"] = _build_program()
    return _CACHE["nc"]


def _make_mask():
    # msk[jj, z] = 1 if z >= jj + 384 else 0; diagonal-position-p mask
    # tile is msk[:, 384-128p : 384-128p+512].
    jj = np.arange(128)[:, None]
    z = np.arange(896)[None, :]
    return (z >= jj + 384).astype(np.float16)


def _make_in_maps(x, w_qkv, w_out):
    mask = _make_mask()
    in_maps = []
    for core in range(NCORES):
        b, hg = core // 2, core % 2
        cs = slice(hg * 512, (hg + 1) * 512)
        f16 = np.float16
        in_maps.append(
            {
                "xt": np.ascontiguousarray(x[b].T).astype(f16),
                "wq": np.ascontiguousarray(
                    w_qkv[:, hg * 512 : hg * 512 + 512]
                ).astype(f16),
                "wk": np.ascontiguousarray(
                    w_qkv[:, 1024 + hg * 512 : 1024 + hg * 512 + 512]
                ).astype(f16),
                "wv": np.ascontiguousarray(
                    w_qkv[:, 2048 + hg * 512 : 2048 + hg * 512 + 512]
                ).astype(f16),
                "wo": np.ascontiguousarray(w_out[cs, :]).astype(f16),
                "msk": mask,
            }
        )
    return in_maps


def _run_device(in_maps, trace=False):
    from concourse.bass_utils import run_bass_kernel_spmd

    nc = _get_program()
    return run_bass_kernel_spmd(
        nc, in_maps, core_ids=list(range(NCORES)), trace=trace
    )


def kernel(x, w_qkv, w_out, b_out):
    x = np.asarray(x, dtype=np.float32)
    w_qkv = np.asarray(w_qkv, dtype=np.float32)
    w_out = np.asarray(w_out, dtype=np.float32)
    b_out = np.asarray(b_out, dtype=np.float32)

    res = _run_device(_make_in_maps(x, w_qkv, w_out)).results
    out = np.empty((B, T, C), dtype=np.float32)
    for b in range(B):
        out[b] = res[2 * b]["out"] + res[2 * b + 1]["out"] + b_out
    return out


# revision 38
# speedup vs baseline: 1.0262x; 1.0262x over previous
"""Multi-head causal self-attention on 8 Trainium2 NeuronCores.

Problem: x[4,2048,1024] @ w_qkv[1024,3072] -> 16-head causal attention
         -> @ w_out[1024,1024] + b_out.

Sharding (hardcoded): 8 cores = 4 batches x 2 head-groups of 8 heads.
Core c handles batch b = c//2 and heads hg*8..hg*8+8, hg = c%2.
Each core computes a partial output [2048,1024] (its 8 heads pushed
through its w_out row-slice); host sums the two head-group partials per
batch and adds b_out.

Everything computes in fp16 (10 mantissa bits; fp32 PSUM accumulation),
which runs matmuls at full 1 cycle/row PE rate and lands ~7e-4 relative
error vs the fp32 reference.

Device algorithm per core (all "transposed orientation" so the only
transpose needed -- x^T -- is done for free on the host):
  qT/kT [512, 2048] and v (natural [2048, 512]) via fp16 matmuls.
  Per head pair (2 heads = 128 partitions), per 512-wide query chunk:
    scores^T[j,i] for both heads into one 2-bank PSUM tile via
    row-tiled (K=64) matmul pairs; ONE exp per key-tile on ScalarE
    (p^T fp16); causal masking via a precomputed 0/1 mask multiply on
    the diagonal band plus variable-width (narrowed) tiles;
    out^T[d,i] += col-tiled matmuls (PSUM accum over j),
    denom[i]   += ones-vector matmuls (M=1) into shared denom banks
    (4 col-strip rows per bank, zero-established by a dummy matmul).
  att^T (unnormalized) is copied to SBUF immediately (frees PSUM);
  1/denom via one batched DVE reciprocal per bank, broadcast over
  partitions via a DRAM bounce, then in-place multiply into att^T.
  partial = att^T.T @ w_out_slice -> DMA to DRAM.

Emission is software-pipelined per 512-token stage s: QKV(s),
out-projection(s-1), attention(s), so the Tile scheduler overlaps
PE-heavy projection work with ScalarE-heavy softmax work and hides the
softmax-denominator normalization latency.
"""

import os
import sys

import numpy as np

if "/opt/trn_rl_repo" not in sys.path:
    sys.path.insert(0, "/opt/trn_rl_repo")

B, T, C = 4, 2048, 1024
H, D = 16, 64
NCORES = 8
HPC = 8  # heads per core
PAIRS = 4  # head pairs per core
CCH = 8  # contraction chunks over C (1024/128)
ICH = 4  # i (query) chunks of 512
NJT = 16  # j (key) tiles of 128

_CACHE = {}


def _build_program():
    import concourse.mybir as mybir
    import concourse.tile as tile
    from concourse import bacc

    f32 = mybir.dt.float32
    f32r = mybir.dt.float32r
    bf16 = mybir.dt.bfloat16
    f16 = mybir.dt.float16
    EXP = mybir.ActivationFunctionType.Exp

    nc = bacc.Bacc(
        "TRN2", target_bir_lowering=False, debug=False, num_devices=NCORES
    )
    xt = nc.dram_tensor("xt", [C, T], f16, kind="ExternalInput").ap()
    wq = nc.dram_tensor("wq", [C, 512], f16, kind="ExternalInput").ap()
    wk = nc.dram_tensor("wk", [C, 512], f16, kind="ExternalInput").ap()
    wv = nc.dram_tensor("wv", [C, 512], f16, kind="ExternalInput").ap()
    wo = nc.dram_tensor("wo", [512, C], f16, kind="ExternalInput").ap()
    msk = nc.dram_tensor("msk", [128, 896], f16, kind="ExternalInput").ap()
    out = nc.dram_tensor("out", [T, C], f32, kind="ExternalOutput").ap()

    with tile.TileContext(nc) as tc:
        with (
            tc.tile_pool(name="wpool", bufs=16) as wpool,
            tc.tile_pool(name="wvpool", bufs=8) as wvpool,
            tc.tile_pool(name="wopool", bufs=4) as wopool,
            tc.tile_pool(name="xpool", bufs=8) as xpool,
            tc.tile_pool(name="qkpool", bufs=8) as qkpool,
            tc.tile_pool(name="vpool", bufs=16) as vpool,
            tc.tile_pool(name="apool", bufs=4) as apool,
            tc.tile_pool(name="ppool", bufs=12) as ppool,
            tc.tile_pool(name="cpool", bufs=1) as cpool,
            tc.tile_pool(name="rpool", bufs=4) as rpool,
            tc.tile_pool(name="qpool", bufs=4) as qpool,
            tc.tile_pool(name="opool", bufs=4) as opool,
            tc.tile_pool(name="dpool", bufs=4, space="DRAM") as dpool,
            tc.tile_pool(name="ps_a", bufs=2, space="PSUM") as ps_a,
            tc.tile_pool(name="ps_s", bufs=2, space="PSUM") as ps_s,
            tc.tile_pool(name="ps_o", bufs=1, space="PSUM") as ps_o,
            tc.tile_pool(name="ps_d", bufs=1, space="PSUM") as ps_d,
        ):
            # ---- constants / weights resident in SBUF ----
            mask_sb = cpool.tile([128, 896], f16, name="mask_sb")
            nc.sync.dma_start(out=mask_sb, in_=msk)
            ones_sb = cpool.tile([128, 1], f16, name="ones_sb")
            nc.vector.memset(ones_sb, 1.0)
            zer_sb = cpool.tile([128, 128], f16, name="zer_sb")
            nc.vector.memset(zer_sb, 0.0)

            w_sb = {}

            def load_w(wname, wap):
                for cc in range(CCH):
                    t = wpool.tile(
                        [128, 512], f16, name=f"{wname}_{cc}", tag="w"
                    )
                    nc.sync.dma_start(
                        out=t, in_=wap[cc * 128 : (cc + 1) * 128, :]
                    )
                    w_sb[wname, cc] = t

            # First compute needs wq + x^T: queue those DMAs first so the
            # PE starts early. x^T loads as 8 big 512KB transfers (whole
            # token range per contraction chunk).
            load_w("wq", wq)
            xt_sb = []
            for cc in range(CCH):
                xt_t = xpool.tile([128, T], f16, name=f"xt_{cc}", tag="xt")
                nc.sync.dma_start(out=xt_t, in_=xt[cc * 128 : (cc + 1) * 128, :])
                xt_sb.append(xt_t)
            load_w("wk", wk)
            for cc in range(CCH):
                t = wvpool.tile([128, 512], f16, name=f"wv_{cc}", tag="wv")
                nc.sync.dma_start(out=t, in_=wv[cc * 128 : (cc + 1) * 128, :])
                w_sb["wv", cc] = t
            wo_sb = []
            for fc in range(4):
                t = wopool.tile([128, C], f16, name=f"wo_{fc}", tag="wo")
                nc.sync.dma_start(out=t, in_=wo[fc * 128 : (fc + 1) * 128, :])
                wo_sb.append(t)

            # ---- persistent activations ----
            qT = [
                qkpool.tile([128, T], f16, name=f"qT_{p}", tag="qk")
                for p in range(PAIRS)
            ]
            kT = [
                qkpool.tile([128, T], f16, name=f"kT_{p}", tag="qk")
                for p in range(PAIRS)
            ]
            v_sb = [
                vpool.tile([128, 512], f16, name=f"v_{j}", tag="v")
                for j in range(NJT)
            ]
            att = [
                apool.tile([128, T], f16, name=f"att_{p}", tag="att")
                for p in range(PAIRS)
            ]

            def phase_a(t4):
                """QKV projections for token chunk t4 (512 tokens)."""
                tsl4 = slice(t4 * 512, (t4 + 1) * 512)
                xts = [xt_sb[cc][:, tsl4] for cc in range(CCH)]
                for wname, dst in (("wq", qT), ("wk", kT)):
                    for n in range(PAIRS):
                        ps = ps_a.tile([128, 512], f32, name="ps_qk", tag="psA")
                        for cc in range(CCH):
                            nc.tensor.matmul(
                                ps,
                                lhsT=w_sb[wname, cc][:, n * 128 : (n + 1) * 128],
                                rhs=xts[cc][:],
                                start=(cc == 0),
                                stop=(cc == CCH - 1),
                            )
                        nc.vector.tensor_copy(
                            dst[n][:, t4 * 512 : (t4 + 1) * 512], ps
                        )
                for tt in range(4):
                    ps = ps_a.tile([128, 512], f32, name="ps_v", tag="psA")
                    for cc in range(CCH):
                        nc.tensor.matmul(
                            ps,
                            lhsT=xts[cc][:, tt * 128 : (tt + 1) * 128],
                            rhs=w_sb["wv", cc][:],
                            start=(cc == 0),
                            stop=(cc == CCH - 1),
                        )
                    nc.vector.tensor_copy(v_sb[t4 * 4 + tt], ps)

            def phase_b(ic):
                """Attention for query chunk ic (512 queries), all pairs."""
                isl = slice(ic * 512, (ic + 1) * 512)
                njt = 4 * ic + 4
                # Two denominator banks per ic: bank A rows {0,32,64,96} =
                # pairs 0,1; bank B = pairs 2,3. One zeroing matmul each
                # establishes the group and write-ordering.
                dbanks = []
                for g in range(2):
                    bank = ps_d.tile([128, 512], f32, name=f"ps_den{g}", tag="psd")
                    nc.tensor.matmul(
                        bank,
                        lhsT=zer_sb,
                        rhs=mask_sb[:, 0:512],
                        start=True,
                        stop=False,
                        skip_group_check=True,
                    )
                    dbanks.append(bank)

                def norm_group(g):
                    """1/denominators for pairs 2g, 2g+1 -> rdb + in-place mul."""
                    bank = dbanks[g]
                    rec = rpool.tile([128, 1024], f32, name="rec", tag="rec")
                    nc.vector.tensor_copy(rec[0:97, 0:512], bank[0:97, :])
                    nc.vector.reciprocal(rec[0:97, 512:1024], rec[0:97, 0:512])
                    dsc = dpool.tile([4, 512], f32, name="dsc", tag="dsc")
                    for r in range(4):
                        nc.sync.dma_start(
                            out=dsc[r : r + 1, :],
                            in_=rec[32 * r : 32 * r + 1, 512:1024],
                        )
                    for lp in range(2):
                        pr = 2 * g + lp
                        rdb = rpool.tile([128, 512], f32, name="rdb", tag="rdb")
                        nc.sync.dma_start(
                            out=rdb[0:64, :],
                            in_=dsc[2 * lp : 2 * lp + 1, :].broadcast_to([64, 512]),
                        )
                        nc.sync.dma_start(
                            out=rdb[64:128, :],
                            in_=dsc[2 * lp + 1 : 2 * lp + 2, :].broadcast_to(
                                [64, 512]
                            ),
                        )
                        asl = att[pr][:, isl]
                        nc.vector.tensor_mul(asl, asl, rdb)

                for pr in range(PAIRS):
                    ps_out = ps_o.tile([128, 512], f32, name="ps_out", tag="pso")
                    dbank = dbanks[pr // 2]
                    dp0 = 64 * (pr % 2)
                    dp1 = 64 * (pr % 2) + 32
                    # Zero the whole ps_out bank in one matmul: establishes
                    # the accumulation group and a WAW dep ordering it before
                    # both col-tiled sub-chains.
                    nc.tensor.matmul(
                        ps_out,
                        lhsT=zer_sb,
                        rhs=mask_sb[:, 0:512],
                        start=True,
                        stop=False,
                        skip_group_check=True,
                    )
                    pacc0 = qpool.tile([128, 512], f16, name="pacc0", tag="pacc")
                    pacc1 = qpool.tile([128, 512], f16, name="pacc1", tag="pacc")
                    for jt in range(njt):
                        jsl = slice(jt * 128, (jt + 1) * 128)
                        dpos = jt - 4 * ic
                        # Causal: query columns below 128*dpos within this
                        # chunk see none of this key tile. fp32r matmuls run
                        # 4 cyc/row under N=256, so don't narrow QK below
                        # that; bf16 pv/den and the exp narrow fully.
                        ioff = 128 * dpos if dpos > 0 else 0
                        qoff = min(ioff, 256)
                        w = 512 - ioff
                        islw = slice(ic * 512 + ioff, (ic + 1) * 512)
                        islq = slice(ic * 512 + qoff, (ic + 1) * 512)
                        sb = ps_s.tile([128, 1024], f32, name="sb", tag="pss")
                        nc.tensor.matmul(
                            sb[:, qoff:512],
                            lhsT=kT[pr][0:64, jsl],
                            rhs=qT[pr][0:64, islq],
                            start=True,
                            stop=True,
                            tile_position=(0, 0),
                        )
                        nc.tensor.matmul(
                            sb[:, 512:1024],
                            lhsT=kT[pr][64:128, jsl],
                            rhs=qT[pr][64:128, isl],
                            start=True,
                            stop=True,
                            tile_position=(64, 0),
                        )
                        pTb = ppool.tile([128, 1024], f16, name="pTb", tag="pT")
                        # One exp covers both heads; the dead zone between
                        # the halves on diagonal tiles holds stale-but-finite
                        # scores and is never read downstream.
                        nc.scalar.activation(
                            pTb[:, ioff:1024], sb[:, ioff:1024], EXP, scale=0.125
                        )
                        pT0 = pTb[:, 0:512]
                        pT1 = pTb[:, 512:1024]
                        if dpos >= 0:
                            msl = mask_sb[:, 384 : 384 + w]
                            nc.vector.tensor_mul(
                                pT0[:, ioff:512], pT0[:, ioff:512], msl
                            )
                            nc.vector.tensor_mul(
                                pT1[:, ioff:512], pT1[:, ioff:512], msl
                            )
                        last = jt == njt - 1
                        vt = v_sb[jt]
                        nc.tensor.matmul(
                            ps_out[0:64, ioff:512],
                            lhsT=vt[:, pr * 128 : pr * 128 + 64],
                            rhs=pT0[:, ioff:512],
                            start=False,
                            stop=False,
                            tile_position=(0, 0),
                            skip_group_check=True,
                        )
                        nc.tensor.matmul(
                            ps_out[64:128, ioff:512],
                            lhsT=vt[:, pr * 128 + 64 : pr * 128 + 128],
                            rhs=pT1[:, ioff:512],
                            start=False,
                            stop=last,
                            tile_position=(0, 64),
                            skip_group_check=True,
                        )
                        if jt == 0:
                            nc.vector.tensor_copy(pacc0, pT0)
                            nc.vector.tensor_copy(pacc1, pT1)
                        else:
                            nc.vector.tensor_add(
                                pacc0[:, ioff:512],
                                pacc0[:, ioff:512],
                                pT0[:, ioff:512],
                            )
                            nc.vector.tensor_add(
                                pacc1[:, ioff:512],
                                pacc1[:, ioff:512],
                                pT1[:, ioff:512],
                            )
                    # Partition-reduce the accumulated p-sums into the
                    # shared denominator bank (2 matmuls instead of 2/key-tile).
                    nc.tensor.matmul(
                        dbank[dp0 : dp0 + 1, :],
                        lhsT=ones_sb,
                        rhs=pacc0,
                        start=False,
                        stop=False,
                        tile_position=(0, dp0),
                        skip_group_check=True,
                    )
                    nc.tensor.matmul(
                        dbank[dp1 : dp1 + 1, :],
                        lhsT=ones_sb,
                        rhs=pacc1,
                        start=False,
                        stop=False,
                        tile_position=(0, dp1),
                        skip_group_check=True,
                    )
                    # Unnormalized copy frees ps_out quickly; normalization
                    # happens in-place on att once the broadcast lands.
                    asl = att[pr][:, isl]
                    nc.vector.tensor_copy(asl, ps_out)
                    if pr % 2 == 1:
                        norm_group(pr // 2)

            def phase_c(s):
                """Output projection for token tiles 4s..4s+4."""
                for tt in range(4 * s, 4 * s + 4):
                    tsl = slice(tt * 128, (tt + 1) * 128)
                    for n in range(2):
                        ps = ps_a.tile([128, 512], f32, name="ps_c", tag="psA")
                        for fc in range(4):
                            nc.tensor.matmul(
                                ps,
                                lhsT=att[fc][:, tsl],
                                rhs=wo_sb[fc][:, n * 512 : (n + 1) * 512],
                                start=(fc == 0),
                                stop=(fc == 3),
                            )
                        ost = opool.tile([128, 512], f32, name="ost", tag="ost")
                        nc.vector.tensor_copy(ost, ps)
                        nc.sync.dma_start(
                            out=out[tsl, n * 512 : (n + 1) * 512], in_=ost
                        )

            for s in range(4):
                phase_a(s)
                if s >= 1:
                    phase_c(s - 1)
                phase_b(s)
            phase_c(3)

    nc.compile()
    return nc


def _get_program():
    if "nc" not in _CACHE:
        _CACHE["nc"] = _build_program()
    return _CACHE["nc"]


def _make_mask():
    # msk[jj, z] = 1 if z >= jj + 384 else 0; diagonal-position-p mask
    # tile is msk[:, 384-128p : 384-128p+512].
    jj = np.arange(128)[:, None]
    z = np.arange(896)[None, :]
    return (z >= jj + 384).astype(np.float16)


def _make_in_maps(x, w_qkv, w_out):
    mask = _make_mask()
    in_maps = []
    for core in range(NCORES):
        b, hg = core // 2, core % 2
        cs = slice(hg * 512, (hg + 1) * 512)
        f16 = np.float16
        in_maps.append(
            {
                "xt": np.ascontiguousarray(x[b].T).astype(f16),
                "wq": np.ascontiguousarray(
                    w_qkv[:, hg * 512 : hg * 512 + 512]
                ).astype(f16),
                "wk": np.ascontiguousarray(
                    w_qkv[:, 1024 + hg * 512 : 1024 + hg * 512 + 512]
                ).astype(f16),
                "wv": np.ascontiguousarray(
                    w_qkv[:, 2048 + hg * 512 : 2048 + hg * 512 + 512]
                ).astype(f16),
                "wo": np.ascontiguousarray(w_out[cs, :]).astype(f16),
                "msk": mask,
            }
        )
    return in_maps


def _run_device(in_maps, trace=False):
    from concourse.bass_utils import run_bass_kernel_spmd

    nc = _get_program()
    return run_bass_kernel_spmd(
        nc, in_maps, core_ids=list(range(NCORES)), trace=trace
    )


def kernel(x, w_qkv, w_out, b_out):
    x = np.asarray(x, dtype=np.float32)
    w_qkv = np.asarray(w_qkv, dtype=np.float32)
    w_out = np.asarray(w_out, dtype=np.float32)
    b_out = np.asarray(b_out, dtype=np.float32)

    res = _run_device(_make_in_maps(x, w_qkv, w_out)).results
    out = np.empty((B, T, C), dtype=np.float32)
    for b in range(B):
        out[b] = res[2 * b]["out"] + res[2 * b + 1]["out"] + b_out
    return out
